# revision 1
# baseline (speedup 1.0000x reference)
"""PPO actor loss (GAE + clipped surrogate + entropy) on 8 Trainium2 NeuronCores.

Self-contained Bass/Tile kernel for the nn_Actor problem:
    out = mean(min(adv * ratio, adv * 0.8)) + (log(sigma) + 0.5 + 0.5*log(2*pi))

Sharding: data-parallel over the batch (environment) axis B=512 -> 64 envs per
core. Per core, the [64, 2048, 16] elementwise tensors are viewed as
[128, 16384] (partition p = 2*env + time_half). The GAE recurrence runs as a
single hardware `tensor_tensor_scan` over reversed-time APs. The final mean is
combined across cores with two tiny AllReduces (the first one overlaps the
main loop). Entropy stays SBUF-resident; the output is entropy + mean.
"""
import numpy as np
import concourse.bacc as bacc
import concourse.tile as tile
from concourse import mybir
from concourse.bass_utils import run_bass_kernel_spmd

F32 = mybir.dt.float32
BF16 = mybir.dt.bfloat16
I32 = mybir.dt.int32
AF = mybir.ActivationFunctionType
OP = mybir.AluOpType

# Pin all activations to the one ACT table set that contains ln+exp+square+
# identity; otherwise ln/exp alternation reloads ACT tables (~1.3us each).
_orig_get_act_tables = bacc.get_activation_tables


def _pinned_act_tables(arch):
    tabs = _orig_get_act_tables(arch)
    return {
        name: (funcs if name == "natural_log_exp_and_others" else set())
        for name, funcs in tabs.items()
    }


bacc.get_activation_tables = _pinned_act_tables

N_CORES = 8
B, T, A = 512, 2048, 16
BL = B // N_CORES            # 64 envs per core
P = 128
FT = T * A // 2              # 16384 flat free elems per partition
GAMMA, LAM, CLIP = 0.99, 0.95, 0.2
C_ENT = float(0.5 + 0.5 * np.log(2.0 * np.pi))
K_ENT = float(np.exp(C_ENT))
NTOT = float(B * T * A)


def build_nc(chunk_f=2048, input_bufs=2, temp_bufs=2, split_ar=4,
             bf16_tail=True):
    F = chunk_f
    NCH = FT // F
    TW = F // A                  # t-steps covered per chunk (per partition)

    nc = bacc.Bacc("TRN2", target_bir_lowering=False, debug=False)

    rewards = nc.declare_dram_parameter("rewards", [BL, T], F32, isOutput=False)
    critic = nc.declare_dram_parameter("critic_out", [BL, T + 1], F32,
                                       isOutput=False)
    dones = nc.declare_dram_parameter("dones", [BL, T], I32, isOutput=False)
    big = {}
    for name in ("mu", "sigma", "old_mu", "old_sigma", "actions"):
        big[name] = nc.declare_dram_parameter(name, [BL, T * A], F32,
                                              isOutput=False)
    out_p = nc.declare_dram_parameter("out", [BL, T * A], F32, isOutput=True)

    bv = {k: v[:].rearrange("b (h f) -> (b h) f", h=2) for k, v in big.items()}
    out_v = out_p[:].rearrange("b (h f) -> (b h) f", h=2)

    n_ar1 = min(split_ar, NCH) if split_ar else NCH

    with tile.TileContext(nc) as tc:
        with (
            tc.tile_pool(name="dram", bufs=1, space="DRAM") as dram,
            tc.tile_pool(name="persist", bufs=1) as persist,
            tc.tile_pool(name="psum", bufs=1, space="PSUM") as psum,
        ):
            adv_scr = dram.tile([BL, T], F32)
            cc_in1 = dram.tile([1, 8], F32)
            cc_out1 = dram.tile([1, 8], F32)
            cc_in2 = dram.tile([1, 8], F32)
            cc_out2 = dram.tile([1, 8], F32)

            ls_res = persist.tile([P, FT], F32)      # entropy, resident
            adv128 = persist.tile([P, T // 2], F32)
            a8 = None if bf16_tail else persist.tile([P, T // 2], F32)
            acc = persist.tile([P, NCH], F32)        # per-chunk partials
            r1 = persist.tile([P, 1], F32)
            r2 = persist.tile([P, 1], F32)
            ccol = persist.tile([P, 1], F32)
            ones_col = persist.tile([P, 1], F32)
            mean_col = persist.tile([P, 1], F32)
            mc2 = persist.tile([P, 1], F32)
            s_loc1 = persist.tile([1, 8], F32)
            s_loc2 = persist.tile([1, 8], F32)
            ps1 = psum.tile([1, 1], F32)
            ps2 = psum.tile([1, 1], F32)

            nc.vector.memset(ccol[:], C_ENT)
            nc.vector.memset(ones_col[:], 1.0)
            nc.vector.memset(s_loc1[:], 0.0)
            nc.vector.memset(s_loc2[:], 0.0)

            with (
                tc.tile_pool(name="inp", bufs=input_bufs) as ip,
                tc.tile_pool(name="tmp", bufs=temp_bufs) as tp,
                tc.tile_pool(name="tmp1", bufs=1) as tp1,
            ):
                # ------------- Phase A: GAE scan (reuses tmp tags) ---------
                rw = tp.tile([BL, T], F32, tag="t4")
                cr = tp.tile([BL, T + 1], F32, tag="t1")
                dn = tp.tile([BL, T], I32, tag="t1")
                nc.sync.dma_start(rw[:], rewards[:])
                nc.sync.dma_start(cr[:], critic[:])
                nc.sync.dma_start(dn[:], dones[:])

                t1c = tp.tile([BL, T], F32, tag="t4")
                nc.vector.scalar_tensor_tensor(
                    t1c[:], cr[:, 1:T + 1], GAMMA, rw[:], OP.mult, OP.add)
                a_n = tp.tile([BL, T], F32, tag="t4")
                nc.vector.tensor_tensor(a_n[:], t1c[:], cr[:, 0:T],
                                        OP.subtract)
                nc.vector.memset(a_n[:, T - 1:T], 0.0)
                # q[t] = g*lam*(1 - d[t+1]) on ScalarE: Copy(-gl*d + gl)
                q_n = tp1.tile([BL, T], F32, tag="t5")
                nc.scalar.activation(q_n[:, 0:T - 1], dn[:, 1:T], AF.Copy,
                                     scale=-GAMMA * LAM,
                                     bias=float(GAMMA * LAM))
                nc.vector.memset(q_n[:, T - 1:T], 0.0)
                # reversed-time linear recurrence: adv = td + q * adv_next
                adv_n = tp.tile([BL, T], F32, tag="t1")
                nc.vector.tensor_tensor_scan(
                    adv_n[:, ::-1], q_n[:, ::-1], a_n[:, ::-1], 0.0,
                    OP.mult, OP.add)
                # remap [64, 2048] -> [128, 1024] via DRAM bounce
                nc.gpsimd.dma_start(adv_scr[:], adv_n[:])
                nc.gpsimd.dma_start(
                    adv128[:], adv_scr[:].rearrange("b (h j) -> (b h) j", h=2))
                if not bf16_tail:
                    nc.vector.tensor_scalar_mul(a8[:], adv128[:], 1.0 - CLIP)

                def emit_partial_ar(lo, hi, r_t, ps_t, s_loc_t, cc_in_t,
                                    cc_out_t):
                    nc.vector.tensor_reduce(
                        r_t[:], acc[:, lo:hi], mybir.AxisListType.X, OP.add)
                    nc.tensor.matmul(ps_t[:], ones_col[:], r_t[:])
                    nc.scalar.copy(s_loc_t[:, 0:1], ps_t[:])
                    nc.gpsimd.dma_start(cc_in_t[:], s_loc_t[:])
                    nc.gpsimd.collective_compute(
                        "AllReduce", OP.add,
                        replica_groups=[list(range(N_CORES))],
                        ins=[cc_in_t.opt()],
                        outs=[cc_out_t.opt()],
                    )

                # ------------- Phase B: streamed objective -----------------
                for c in range(NCH):
                    cs = slice(c * F, (c + 1) * F)
                    a_c = ip.tile([P, F], F32, tag="a")
                    m_c = ip.tile([P, F], F32, tag="m")
                    s_c = ip.tile([P, F], F32, tag="s")
                    om_c = ip.tile([P, F], F32, tag="om")
                    os_c = ip.tile([P, F], F32, tag="os")
                    nc.sync.dma_start(a_c[:], bv["actions"][:, cs])
                    nc.sync.dma_start(m_c[:], bv["mu"][:, cs])
                    nc.sync.dma_start(s_c[:], bv["sigma"][:, cs])
                    nc.sync.dma_start(om_c[:], bv["old_mu"][:, cs])
                    nc.sync.dma_start(os_c[:], bv["old_sigma"][:, cs])

                    ent = ls_res[:, cs]
                    # t4 chain: dm -> z -> z2 -> v -> x
                    t4 = tp.tile([P, F], F32, tag="t4")
                    nc.vector.tensor_tensor(t4[:], a_c[:], m_c[:], OP.subtract)
                    t5 = tp1.tile([P, F], F32, tag="t5")
                    nc.vector.tensor_tensor(t5[:], a_c[:], om_c[:],
                                            OP.subtract)
                    nc.scalar.activation(ent, s_c[:], AF.Ln, scale=K_ENT)
                    # rs = 1/sigma, overwrites sigma input tile
                    nc.scalar.activation(s_c[:], ent, AF.Exp, scale=-1.0,
                                         bias=ccol[:, 0:1])
                    ento = tp.tile([P, F], F32, tag="t1")
                    nc.scalar.activation(ento[:], os_c[:], AF.Ln, scale=K_ENT)
                    nc.scalar.activation(os_c[:], ento[:], AF.Exp, scale=-1.0,
                                         bias=ccol[:, 0:1])
                    nc.vector.tensor_tensor(t4[:], t4[:], s_c[:], OP.mult)
                    nc.vector.tensor_tensor(t5[:], t5[:], os_c[:], OP.mult)
                    nc.scalar.activation(t4[:], t4[:], AF.Square)
                    nc.scalar.activation(t5[:], t5[:], AF.Square)
                    nc.vector.scalar_tensor_tensor(
                        t4[:], ent, 2.0, t4[:], OP.mult, OP.add)
                    nc.vector.scalar_tensor_tensor(
                        t5[:], ento[:], 2.0, t5[:], OP.mult, OP.add)
                    nc.vector.tensor_tensor(t4[:], t5[:], t4[:], OP.subtract)

                    advv = adv128[:, c * TW:(c + 1) * TW].unsqueeze(2) \
                        .broadcast_to([P, TW, A])
                    if bf16_tail:
                        # ratio bf16; adv broadcast materialized on ScalarE
                        ratio = tp.tile([P, F], BF16, tag="t6")
                        nc.scalar.activation(ratio[:], t4[:], AF.Exp,
                                             scale=0.5)
                        advb = tp.tile([P, F], BF16, tag="tadv")
                        nc.scalar.activation(
                            advb[:].rearrange("p (w a) -> p w a", a=A), advv,
                            AF.Copy)
                        nc.vector.tensor_tensor(ratio[:], ratio[:], advb[:],
                                                OP.mult)
                        # 0.8*adv in bf16, carved out of the dead t5 region
                        a8b = t5[:, 0:F // 2].bitcast(BF16)
                        nc.vector.tensor_scalar_mul(a8b, advb[:], 1.0 - CLIP)
                        nc.vector.tensor_tensor(ratio[:], ratio[:], a8b,
                                                OP.min)
                        nc.scalar.activation(advb[:], ratio[:], AF.Identity,
                                             accum_out=acc[:, c:c + 1])
                    else:
                        nc.scalar.activation(t4[:], t4[:], AF.Exp, scale=0.5)
                        a8v = a8[:, c * TW:(c + 1) * TW].unsqueeze(2) \
                            .broadcast_to([P, TW, A])
                        t4_3 = t4[:].rearrange("p (w a) -> p w a", a=A)
                        nc.vector.tensor_tensor(t4_3, t4_3, advv, OP.mult)
                        nc.vector.tensor_tensor(t4_3, t4_3, a8v, OP.min)
                        nc.scalar.activation(t5[:], t4[:], AF.Identity,
                                             accum_out=acc[:, c:c + 1])

                    if c == n_ar1 - 1 and n_ar1 < NCH:
                        emit_partial_ar(0, n_ar1, r1, ps1, s_loc1,
                                        cc_in1, cc_out1)

                # ------------- Phase C: reduce + all-reduce ----------------
                if n_ar1 < NCH:
                    emit_partial_ar(n_ar1, NCH, r2, ps2, s_loc2,
                                    cc_in2, cc_out2)
                    nc.gpsimd.dma_start(
                        mean_col[:],
                        cc_out1[:][0:1, 0:1].partition_broadcast(P))
                    nc.gpsimd.dma_start(
                        mc2[:], cc_out2[:][0:1, 0:1].partition_broadcast(P))
                    nc.vector.tensor_tensor(mean_col[:], mean_col[:], mc2[:],
                                            OP.add)
                else:
                    emit_partial_ar(0, NCH, r1, ps1, s_loc1, cc_in1, cc_out1)
                    nc.gpsimd.dma_start(
                        mean_col[:],
                        cc_out1[:][0:1, 0:1].partition_broadcast(P))
                nc.vector.tensor_scalar_mul(mean_col[:], mean_col[:],
                                            1.0 / NTOT)

                # ------------- Phase D: out = entropy + mean ---------------
                for c in range(NCH):
                    cs = slice(c * F, (c + 1) * F)
                    o_c = tp.tile([P, F], F32, tag="t4")
                    nc.vector.tensor_scalar_add(o_c[:], ls_res[:, cs],
                                                mean_col[:, 0:1])
                    nc.sync.dma_start(out_v[:, cs], o_c[:])

    nc.compile()
    return nc


def shard_inputs(rewards, critic_out, mu, sigma, old_mu, old_sigma, actions,
                 dones):
    maps = []
    for i in range(N_CORES):
        s = slice(i * BL, (i + 1) * BL)
        maps.append({
            "rewards": np.ascontiguousarray(rewards[s], np.float32),
            "critic_out": np.ascontiguousarray(critic_out[s], np.float32),
            "dones": np.ascontiguousarray(dones[s], np.int32),
            "mu": np.ascontiguousarray(
                np.asarray(mu)[s].reshape(BL, T * A), np.float32),
            "sigma": np.ascontiguousarray(
                np.asarray(sigma)[s].reshape(BL, T * A), np.float32),
            "old_mu": np.ascontiguousarray(
                np.asarray(old_mu)[s].reshape(BL, T * A), np.float32),
            "old_sigma": np.ascontiguousarray(
                np.asarray(old_sigma)[s].reshape(BL, T * A), np.float32),
            "actions": np.ascontiguousarray(
                np.asarray(actions)[s].reshape(BL, T * A), np.float32),
        })
    return maps


_NC_CACHE = {}


def _get_nc(**build_kw):
    key = tuple(sorted(build_kw.items()))
    if key not in _NC_CACHE:
        _NC_CACHE[key] = build_nc(**build_kw)
    return _NC_CACHE[key]


def kernel_traced(rewards, critic_out, mu, sigma, old_mu, old_sigma, actions,
                  dones, trace=False, **build_kw):
    """Run and return (full_output, BassKernelResults)."""
    nc = _get_nc(**build_kw)
    maps = shard_inputs(rewards, critic_out, mu, sigma, old_mu, old_sigma,
                        actions, dones)
    res = run_bass_kernel_spmd(nc, maps, core_ids=list(range(N_CORES)),
                               trace=trace)
    out = np.concatenate([res.results[i]["out"] for i in range(N_CORES)],
                         axis=0)
    return out.reshape(B, T, A).astype(np.float32), res


def kernel(rewards, critic_out, mu, sigma, old_mu, old_sigma, actions, dones):
    out, _ = kernel_traced(rewards, critic_out, mu, sigma, old_mu, old_sigma,
                           actions, dones)
    return out
